# revision 95
# baseline (speedup 1.0000x reference)
"""Trainium2 Bass kernel for nn_Attention_54305566490745 (pooling attention).

Algebraic reduction: the attention uses a single shared learned query per
head, so the whole module collapses to a weighted pooling:

    dots[b,h,n] = scale * ( x[b,:,n] . wq[:,h]  +  (q . pe)[h,n] )
    attn        = softmax_n(dots)
    s[b,h,:]    = sum_n attn[b,h,n] * x[b,:,n]           # pooled x
    out[b,h,:]  = s[b,h,:] @ Wv[:, h*64:(h+1)*64] + bv[h*64:(h+1)*64]

where wq[:,h] = Wk[:, h-block] @ q_h.

v2: single HBM read of x (c-major bf16 only, no transposed second copy —
the serial DMA device is the roofline, so halving its traffic is the big
win vs the two-layout baseline).  The (n, c)-layout copy needed for the
pooling contraction is produced ON-CHIP: the PE transposes x tiles into
PSUM via identity matmuls (stationary loads are free), and Act/DVE
alternate copying the PSUM tiles back to SBUF as bf16.  The dots are
computed TRANSPOSED (dotsT[n,h]: x tiles stationary, tiny wq moving), so
exp runs on a [128, 64] tile and directly emits attnT — no separate
attention transpose.  Softmax sums are 1-column PE matmuls against ones.
The final projection computes only the needed per-head diagonal block,
transposed (stationary Wv tiles, strided s columns moving: 256 moving
columns total), transposes the [64, 64] result back with one fp32
matmul, and fuses normalization (per-(h,b)-row 1/sum) with the bias add
in one DVE scalar_tensor_tensor.

Distribution: data-parallel over batch, 8 batches per core on 8 cores.
HBM traffic per core = 8.4 MiB (one bf16 read of x) + 0.5 MiB Wv; the
steady state runs at the 360 GB/s DMA roofline (~2.9 us per batch) with
the PE/Act/DVE pipeline fully hidden behind it.
"""

import math
import sys

sys.path.insert(0, "/opt/trn_rl_repo")

import numpy as np
import ml_dtypes

import concourse.bass as bass
import concourse.bacc as bacc
import concourse.mybir as mybir
from concourse import tile
from concourse.bass_utils import run_bass_kernel_spmd
from contextlib import ExitStack

BF16 = mybir.dt.bfloat16
F32 = mybir.dt.float32

B, D, HH, WW = 64, 512, 32, 32
N = HH * WW          # 1024
NH, DH = 8, 64
SCALE = DH ** -0.5
NCORES = 8
BPC = B // NCORES    # 8 batches per core
NCI = D // 128       # 4 c-chunks
NJ = N // 128        # 8 n-chunks


def _emit(ctx, tc, t):
    nc = tc.nc
    cst = ctx.enter_context(tc.tile_pool(name="cst", bufs=1))
    xn_pool = ctx.enter_context(tc.tile_pool(name="xn", bufs=4))
    xts_pool = ctx.enter_context(tc.tile_pool(name="xts", bufs=3))
    attn_pool = ctx.enter_context(tc.tile_pool(name="attn", bufs=3))
    tail_pool = ctx.enter_context(tc.tile_pool(name="tail", bufs=1))
    # PSUM: dt 2 + xt 5 + sT 1 = 8 banks exactly; the tail's projection
    # accumulators recycle the dt slots (their batches' recips are done)
    dt_ps = ctx.enter_context(tc.tile_pool(name="dt_ps", bufs=2, space="PSUM"))
    xt_ps = ctx.enter_context(tc.tile_pool(name="xt_ps", bufs=5, space="PSUM"))
    st_ps = ctx.enter_context(tc.tile_pool(name="st_ps", bufs=1, space="PSUM"))

    # ---- constants: wqpe/peqT/i128 packed into ONE load (each HWDGE
    # dispatch costs ~625ns of serial fill time) ----
    cpack = cst.tile([128, 8 * NCI + 8 * NJ + 128], BF16, name="cpack_sb")
    nc.sync.dma_start(cpack[:], t["cpack"])
    wqpe = cpack[:, 0 : 8 * NCI]
    peqT = cpack[:, 8 * NCI : 8 * NCI + 8 * NJ]
    i128 = cpack[:, 8 * NCI + 8 * NJ :]
    ones = cst.tile([128, 1], BF16, name="ones_sb")
    nc.vector.memset(ones[:], 1.0)
    nbias = cst.tile([128, 1], F32, name="nbias_sb")
    nc.vector.memset(nbias[:], -8.0)
    rs8 = cst.tile([8, 8], F32, name="rs8_sb")

    wv = cst.tile([128, NCI * D], BF16, name="wv_sb")
    bvt = cst.tile([64, 64], F32, name="bvt_sb")
    ones_row = cst.tile([1, 64], F32, name="ones_row_sb")
    nc.vector.memset(ones_row[:], 1.0)
    # softmax scales as a (h, b)-ordered row; each batch's scatter hits a
    # stride-8 column set of the single SBUF row
    rsum_row = cst.tile([1, 64], F32, name="rsum_row_sb")
    rsv = rsum_row[:].rearrange("one (h b) -> one h b", b=BPC)
    stsb = tail_pool.tile([128, BPC * 32], BF16, name="stsb")
    osb = tail_pool.tile([64, 64], F32, name="osb")

    # s^T accumulator for all batches: [c(128), 64*ci + 8*b + h]
    st_acc = st_ps.tile([128, NCI * 64], F32, name="st_acc")


    xb = t["xb"]

    # ---- x loads.  Host ships xb pre-shuffled to (b, q, ci, p, nn) so a
    # half-batch load is a contiguous SBUF column range (precise tile deps)
    # with a 3-dim src AP (balances against the 2-dim dst).  b0 loads as
    # quarters to cut pipeline-fill latency. ----
    xns = [None] * BPC
    QN = N // 4  # 256 n-cols per quarter

    def xn_tile(b, ci, j):
        """[128, 128] view of x tile (c-chunk ci, n-chunk j) of batch b."""
        col = 1024 * (j // 2) + QN * ci + 128 * (j % 2)
        return xns[b][:, col : col + 128]

    def stage_load(b):
        xn = xn_pool.tile([128, NCI * N], BF16, name=f"xn{b}", tag="xn")
        nparts = 4 if b == 0 else 2
        rows = 2048 // nparts
        for k in range(nparts):
            src = xb[2048 * b + rows * k : 2048 * b + rows * (k + 1), :].rearrange(
                "(g p) nn -> p g nn", p=128
            )
            cols = rows * QN // 128  # columns this load covers
            dst = xn[:, cols * k : cols * (k + 1)].rearrange(
                "p (g nn) -> p g nn", nn=QN
            )
            nc.sync.dma_start(dst, src)
        xns[b] = xn

    state = {}

    def stage_dots_half(b, hf):
        """transposed dots for n-chunks of one half: dotsT[n, h] chains."""
        if hf == 0:
            dt = dt_ps.tile([128, 512], F32, name=f"dt{b}", tag="dt")
            state[b] = {"dt": dt}
        dt = state[b]["dt"]
        for j in range(4 * hf, 4 * hf + 4):
            o = dt[:, 8 * j : 8 * j + 8]
            # init with the (q . pe) term
            nc.tensor.matmul(o, i128, peqT[:, 8 * j : 8 * j + 8],
                             start=True, stop=False)
            for ci in range(NCI):
                nc.tensor.matmul(
                    o,
                    xn_tile(b, ci, j),
                    wqpe[:, 8 * ci : 8 * ci + 8],
                    start=False,
                    stop=(ci == NCI - 1),
                )

    def stage_trans_half(b, hf, upto=4, skip=0):
        """PE-transpose one half of x into (n, c) PSUM tiles (one n-chunk
        per bank, 5 in flight so copies stream); Act copies even chunks,
        DVE odd."""
        if hf == 0 and skip == 0:
            xts = xts_pool.tile([128, NJ * D], BF16, name=f"xts{b}", tag="xts")
            state[b]["xts"] = xts
        xts = state[b]["xts"]
        for j in range(4 * hf + skip, 4 * hf + upto):
            xt = xt_ps.tile([128, D], F32, name=f"xt{b}_{j}", tag="xt")
            for ci in range(NCI):
                nc.tensor.matmul(
                    xt[:, 128 * ci : 128 * ci + 128],
                    xn_tile(b, ci, j),
                    i128,
                    start=True,
                    stop=True,
                )
            dst = xts[:, D * j : D * (j + 1)]
            if j % 2 == 0:
                nc.scalar.copy(dst, xt[:])
            else:
                nc.vector.tensor_copy(dst, xt[:])

    def stage_exp(b):
        """exp(dotsT) -> attnT directly (shift folded into peqT)."""
        attnT = attn_pool.tile([128, 8 * NJ], BF16, name=f"attnT{b}", tag="attnT")
        # exp(dots - 8): 8 is a safe upper bound on the logits (observed max
        # ~4.3), so no max-reduce is needed; the shift cancels in
        # normalization.  Applied via fp32 bias (folding it into bf16 peqT
        # costs ~0.016 absolute per logit).
        nc.scalar.activation(
            attnT[:], state[b]["dt"][:, 0 : 8 * NJ],
            mybir.ActivationFunctionType.Exp,
            bias=nbias[:],
        )
        state[b]["attnT"] = attnT

    def stage_ssum(b):
        """softmax denominators via 1-col matmuls against ones."""
        dt, attnT = state[b]["dt"], state[b]["attnT"]
        for j in range(NJ):
            nc.tensor.matmul(
                dt[0:8, 64:65],
                attnT[:, 8 * j : 8 * j + 8],
                ones[:],
                start=(j == 0),
                stop=(j == NJ - 1),
            )

    def stage_rsum(b):
        # recips collect as columns of rs8 (engines can't write at a
        # partition offset); a tiny SBUF->SBUF DMA on the otherwise-idle
        # gpsimd ring scatters each into rsum_hb's strided rows (parking
        # there is free).  The last batch goes via SP in the tail.
        nc.vector.reciprocal(rs8[:, b : b + 1], state[b]["dt"][0:8, 64:65])
        if b < BPC - 1:
            nc.gpsimd.dma_start(rsv[:, :, b : b + 1], rs8[:, b : b + 1])

    def stage_pool(b):
        """sT[c, (ci,b,h)] += xT_tile^T @ attnT — 8-col matmuls, x stationary."""
        xts, attnT = state[b]["xts"], state[b]["attnT"]
        for ci in range(NCI):
            o = st_acc[:, 64 * ci + 8 * b : 64 * ci + 8 * b + 8]
            for j in range(NJ):
                nc.tensor.matmul(
                    o,
                    xts[:, D * j + 128 * ci : D * j + 128 * ci + 128],
                    attnT[:, 8 * j : 8 * j + 8],
                    start=(j == 0),
                    stop=(j == NJ - 1),
                )
        del state[b]

    st3 = st_acc[:].rearrange("p (ci q) -> p ci q", q=64)
    sb3 = stsb[:].rearrange("p (ci q) -> p ci q", q=64)

    def stage_tail():
        nc.sync.dma_start(rsv[:, :, BPC - 1 : BPC], rs8[:, BPC - 1 : BPC])
        nc.scalar.copy(stsb[:], st_acc[:])
        # Only the per-head diagonal block of s @ Wv is needed, and each
        # head's rows share the same Wv columns — so compute it TRANSPOSED
        # (stationary Wv tile, moving strided s columns): 256 moving columns
        # instead of 2048, output a tiny [64, 64].
        otp = dt_ps.tile([64, 8 * NH], F32, name="otp", tag="dt")
        sb4 = stsb[:].rearrange("p (ci b h) -> p ci b h", b=BPC, h=8)
        for h in range(NH):
            o = otp[:, 8 * h : 8 * h + 8]
            for ci in range(NCI):
                nc.tensor.matmul(
                    o,
                    wv[:, D * ci + 64 * h : D * ci + 64 * h + 64],
                    sb4[:, ci, :, h : h + 1],  # b = 0..7 of head h, stride 8
                    start=(ci == 0),
                    stop=(ci == NCI - 1),
                )
        # normalization grid 1s^T (x) rsum_row via a 1-partition outer
        # product, so the [64,64] result never needs transposing on-chip —
        # the host unpacks the d-major layout
        rsumG = dt_ps.tile([64, 64], F32, name="rsumG", tag="dt")
        nc.tensor.matmul(rsumG[:], ones_row[:], rsum_row[:], start=True, stop=True)
        rsumG_sb = tail_pool.tile([64, 64], F32, name="rsumG_sb")
        nc.vector.tensor_copy(rsumG_sb[:], rsumG[:])
        # osb = otp * rsumG + bv, fused; only one PSUM operand allowed
        nc.vector.scalar_tensor_tensor(
            osb[:], rsumG_sb[:], 1.0, otp[:],
            mybir.AluOpType.bypass, mybir.AluOpType.mult,
        )
        osb2 = tail_pool.tile([64, 64], F32, name="osb2")
        nc.vector.tensor_add(osb2[:], osb[:], bvt[:])
        nc.sync.dma_start(t["out"], osb2[:])

    # software pipeline; stage k of batch b emitted in iteration b + OFF[k]
    for i in range(BPC + 2):
        if i < BPC:
            stage_load(i)
        if i == BPC - 1:
            # queued right behind the last x load, so the transfers slot in
            # as soon as the x stream drains
            nc.sync.dma_start(wv[:], t["wv"])
            nc.sync.dma_start(bvt[:], t["bvt"])
        if 1 <= i <= BPC:
            b = i - 1
            # half 0 compute first so Act/DVE copies start while half 1 of
            # the batch is still in flight on the DMA ring
            stage_dots_half(b, 0)
            stage_trans_half(b, 0)
        if 2 <= i <= BPC + 1:
            # placed mid-queue: its copies finished last iteration, and it
            # no longer blocks the next batch's half-0 PE work
            stage_pool(i - 2)
        if 1 <= i <= BPC:
            stage_dots_half(b, 1)
            if b == BPC - 1:
                # last batch: exp/recip slot between the copies so neither
                # the attnT nor the rsum scatter gates the tail
                stage_trans_half(b, 1, upto=1)
                stage_exp(b)
                stage_ssum(b)
                stage_rsum(b)
                stage_trans_half(b, 1, skip=1)
            else:
                stage_exp(b)
                stage_trans_half(b, 1)
                stage_ssum(b)
                stage_rsum(b)
        if i == BPC + 1:
            stage_tail()


_BUILT = None


def _build():
    global _BUILT
    if _BUILT is not None:
        return _BUILT
    nc = bacc.Bacc("TRN2", target_bir_lowering=False, debug=False)
    t = {
        "xb": nc.dram_tensor("xb", (BPC * 4 * D, N // 4), BF16, kind="ExternalInput").ap(),
        "cpack": nc.dram_tensor("cpack", (128, 8 * NCI + 8 * NJ + 128), BF16, kind="ExternalInput").ap(),
        "wv": nc.dram_tensor("wv", (128, NCI * D), BF16, kind="ExternalInput").ap(),
        "bvt": nc.dram_tensor("bvt", (64, 64), F32, kind="ExternalInput").ap(),
        "out": nc.dram_tensor("out", (64, 64), F32, kind="ExternalOutput").ap(),
    }
    with tile.TileContext(nc) as tc:
        with ExitStack() as ctx:
            _emit(ctx, tc, t)
    nc.compile()
    _BUILT = (nc, t)
    return _BUILT


def _host_consts(q, Wkv, bkv):
    qh = np.asarray(q, np.float32)[0, :, 0, :]                      # (8, 64)
    Wk = np.asarray(Wkv, np.float32)[:, :D]
    Wv = np.asarray(Wkv, np.float32)[:, D:]
    bv = np.asarray(bkv, np.float32)[D:]

    position = np.arange(N, dtype=np.float32)[:, None]
    div_term = np.exp(
        np.arange(0, DH, 2, dtype=np.float32) * (-(math.log(10000.0) / DH))
    )
    pe = np.zeros((N, DH), np.float32)
    pe[:, 0::2] = np.sin(position * div_term)
    pe[:, 1::2] = np.cos(position * div_term)

    wq = np.einsum("chd,hd->ch", Wk.reshape(D, NH, DH), qh) * SCALE  # (512, 8)
    peq = pe @ (qh * SCALE).T                                        # (1024, 8)

    wqpe = np.zeros((128, 8 * NCI), np.float32)
    for ci in range(NCI):
        wqpe[:, 8 * ci : 8 * ci + 8] = wq[128 * ci : 128 * (ci + 1), :]
    peqT = np.zeros((128, 8 * NJ), np.float32)
    for j in range(NJ):
        peqT[:, 8 * j : 8 * j + 8] = peq[128 * j : 128 * (j + 1), :]

    wv_packed = np.zeros((128, NCI * D), np.float32)
    for ci in range(NCI):
        wv_packed[:, D * ci : D * (ci + 1)] = Wv[128 * ci : 128 * (ci + 1), :]

    cpack = np.concatenate(
        [wqpe, peqT, np.eye(128, dtype=np.float32)], axis=1
    )
    # bias in the d-major out layout: bvt[d, 8h+b] = bv[64h+d]
    bvt = np.repeat(
        bv.reshape(NH, DH).T[:, :, None], BPC, axis=2
    ).reshape(DH, NH * BPC)
    return {
        "cpack": cpack.astype(ml_dtypes.bfloat16),
        "wv": wv_packed.astype(ml_dtypes.bfloat16),
        "bvt": np.ascontiguousarray(bvt).astype(np.float32),
    }


def kernel(x, q, Wkv, bkv, num_heads, **kw):
    assert int(num_heads) == NH
    nc, _ = _build()
    consts = _host_consts(q, Wkv, bkv)

    # device layout (b, q, ci, p, nn): a half-batch load is one contiguous
    # SBUF column range with a 3-dim src access pattern
    xb = np.asarray(x, np.float32).reshape(B, NCI, 128, 4, N // 4)
    xb = xb.transpose(0, 3, 1, 2, 4).astype(ml_dtypes.bfloat16)

    in_maps = []
    for i in range(NCORES):
        m = dict(consts)
        m["xb"] = np.ascontiguousarray(xb[i * BPC : (i + 1) * BPC]).reshape(
            BPC * 4 * D, N // 4
        )
        in_maps.append(m)

    res = run_bass_kernel_spmd(nc, in_maps, core_ids=list(range(NCORES)))

    out = np.zeros((B, NH * DH), np.float32)
    for i in range(NCORES):
        # device layout is d-major: out_dev[d, 8h+b] = out[b, 64h+d]
        shard = res.results[i]["out"].reshape(DH, NH, BPC).transpose(2, 1, 0)
        out[i * BPC : (i + 1) * BPC] = shard.reshape(BPC, NH * DH)
    return out


if __name__ == "__main__":
    _build()
    print("build ok")



# revision 103
# speedup vs baseline: 1.0134x; 1.0134x over previous
"""Trainium2 Bass kernel for nn_Attention_54305566490745 (pooling attention).

Algebraic reduction: the attention uses a single shared learned query per
head, so the whole module collapses to a weighted pooling:

    dots[b,h,n] = scale * ( x[b,:,n] . wq[:,h]  +  (q . pe)[h,n] )
    attn        = softmax_n(dots)
    s[b,h,:]    = sum_n attn[b,h,n] * x[b,:,n]           # pooled x
    out[b,h,:]  = s[b,h,:] @ Wv[:, h*64:(h+1)*64] + bv[h*64:(h+1)*64]

where wq[:,h] = Wk[:, h-block] @ q_h.

v2: single HBM read of x (c-major bf16 only, no transposed second copy —
the serial DMA device is the roofline, so halving its traffic is the big
win vs the two-layout baseline).  The (n, c)-layout copy needed for the
pooling contraction is produced ON-CHIP: the PE transposes x tiles into
PSUM via identity matmuls (stationary loads are free), and Act/DVE
alternate copying the PSUM tiles back to SBUF as bf16.  The dots are
computed TRANSPOSED (dotsT[n,h]: x tiles stationary, tiny wq moving), so
exp runs on a [128, 64] tile and directly emits attnT — no separate
attention transpose.  Softmax sums are 1-column PE matmuls against ones.
The final projection computes only the needed per-head diagonal block,
transposed (stationary Wv tiles, strided s columns moving: 256 moving
columns total), transposes the [64, 64] result back with one fp32
matmul, and fuses normalization (per-(h,b)-row 1/sum) with the bias add
in one DVE scalar_tensor_tensor.

Distribution: data-parallel over batch, 8 batches per core on 8 cores.
HBM traffic per core = 8.4 MiB (one bf16 read of x) + 0.5 MiB Wv; the
steady state runs at the 360 GB/s DMA roofline (~2.9 us per batch) with
the PE/Act/DVE pipeline fully hidden behind it.
"""

import math
import sys

sys.path.insert(0, "/opt/trn_rl_repo")

import numpy as np
import ml_dtypes

import concourse.bass as bass
import concourse.bacc as bacc
import concourse.mybir as mybir
from concourse import tile
from concourse.bass_utils import run_bass_kernel_spmd
from contextlib import ExitStack

BF16 = mybir.dt.bfloat16
F32 = mybir.dt.float32

B, D, HH, WW = 64, 512, 32, 32
N = HH * WW          # 1024
NH, DH = 8, 64
SCALE = DH ** -0.5
NCORES = 8
BPC = B // NCORES    # 8 batches per core
NCI = D // 128       # 4 c-chunks
NJ = N // 128        # 8 n-chunks


def _emit(ctx, tc, t):
    nc = tc.nc
    cst = ctx.enter_context(tc.tile_pool(name="cst", bufs=1))
    xn_pool = ctx.enter_context(tc.tile_pool(name="xn", bufs=4))
    xts_pool = ctx.enter_context(tc.tile_pool(name="xts", bufs=3))
    attn_pool = ctx.enter_context(tc.tile_pool(name="attn", bufs=3))
    tail_pool = ctx.enter_context(tc.tile_pool(name="tail", bufs=1))
    # PSUM: dt 2 + xt 5 + sT 1 = 8 banks exactly; the tail's projection
    # accumulators recycle the dt slots (their batches' recips are done)
    dt_ps = ctx.enter_context(tc.tile_pool(name="dt_ps", bufs=2, space="PSUM"))
    xt_ps = ctx.enter_context(tc.tile_pool(name="xt_ps", bufs=5, space="PSUM"))
    st_ps = ctx.enter_context(tc.tile_pool(name="st_ps", bufs=1, space="PSUM"))

    # ---- constants: wqpe/peqT/i128 packed into ONE load (each HWDGE
    # dispatch costs ~625ns of serial fill time) ----
    cpack = cst.tile([128, 8 * NCI + 8 * NJ + 128], BF16, name="cpack_sb")
    nc.sync.dma_start(cpack[:], t["cpack"])
    wqpe = cpack[:, 0 : 8 * NCI]
    peqT = cpack[:, 8 * NCI : 8 * NCI + 8 * NJ]
    i128 = cpack[:, 8 * NCI + 8 * NJ :]
    ones = cst.tile([128, 1], BF16, name="ones_sb")
    nc.vector.memset(ones[:], 1.0)
    nbias = cst.tile([128, 1], F32, name="nbias_sb")
    nc.vector.memset(nbias[:], -8.0)
    rs8 = cst.tile([8, 8], F32, name="rs8_sb")

    wv = cst.tile([128, NCI * D], BF16, name="wv_sb")
    i64f = cst.tile([64, 64], F32, name="i64f_sb")
    bvh = cst.tile([64, 64], F32, name="bvh_sb")
    # softmax scales in (h, b) row order: each batch's 8 values land on a
    # stride-8 partition set, which is a clean strided DMA scatter
    rsum_hb = cst.tile([64, 1], F32, name="rsum_hb_sb")
    rsv = rsum_hb[:].rearrange("(h b) one -> h b one", b=BPC)
    stsb = tail_pool.tile([128, BPC * 32], BF16, name="stsb")
    otsb = tail_pool.tile([64, 64], F32, name="otsb")
    osb = tail_pool.tile([64, 64], F32, name="osb")

    # s^T accumulator for all batches: [c(128), 64*ci + 8*b + h]
    st_acc = st_ps.tile([128, NCI * 64], F32, name="st_acc")


    xb = t["xb"]

    # ---- x loads.  Host ships xb pre-shuffled to (b, q, ci, p, nn) so a
    # half-batch load is a contiguous SBUF column range (precise tile deps)
    # with a 3-dim src AP (balances against the 2-dim dst).  b0 loads as
    # quarters to cut pipeline-fill latency. ----
    xns = [None] * BPC
    QN = N // 4  # 256 n-cols per quarter

    def xn_tile(b, ci, j):
        """[128, 128] view of x tile (c-chunk ci, n-chunk j) of batch b."""
        col = 1024 * (j // 2) + QN * ci + 128 * (j % 2)
        return xns[b][:, col : col + 128]

    def stage_load(b):
        xn = xn_pool.tile([128, NCI * N], BF16, name=f"xn{b}", tag="xn")
        nparts = 4 if b == 0 else 2
        rows = 2048 // nparts
        for k in range(nparts):
            src = xb[2048 * b + rows * k : 2048 * b + rows * (k + 1), :].rearrange(
                "(g p) nn -> p g nn", p=128
            )
            cols = rows * QN // 128  # columns this load covers
            dst = xn[:, cols * k : cols * (k + 1)].rearrange(
                "p (g nn) -> p g nn", nn=QN
            )
            nc.sync.dma_start(dst, src)
        xns[b] = xn

    state = {}

    def stage_dots_half(b, hf):
        """transposed dots for n-chunks of one half: dotsT[n, h] chains."""
        if hf == 0:
            dt = dt_ps.tile([128, 512], F32, name=f"dt{b}", tag="dt")
            state[b] = {"dt": dt}
        dt = state[b]["dt"]
        for j in range(4 * hf, 4 * hf + 4):
            o = dt[:, 8 * j : 8 * j + 8]
            # init with the (q . pe) term
            nc.tensor.matmul(o, i128, peqT[:, 8 * j : 8 * j + 8],
                             start=True, stop=False)
            for ci in range(NCI):
                nc.tensor.matmul(
                    o,
                    xn_tile(b, ci, j),
                    wqpe[:, 8 * ci : 8 * ci + 8],
                    start=False,
                    stop=(ci == NCI - 1),
                )

    def stage_trans_half(b, hf, upto=4, skip=0):
        """PE-transpose one half of x into (n, c) PSUM tiles (one n-chunk
        per bank, 5 in flight so copies stream); Act copies even chunks,
        DVE odd."""
        if hf == 0 and skip == 0:
            xts = xts_pool.tile([128, NJ * D], BF16, name=f"xts{b}", tag="xts")
            state[b]["xts"] = xts
        xts = state[b]["xts"]
        for j in range(4 * hf + skip, 4 * hf + upto):
            xt = xt_ps.tile([128, D], F32, name=f"xt{b}_{j}", tag="xt")
            for ci in range(NCI):
                nc.tensor.matmul(
                    xt[:, 128 * ci : 128 * ci + 128],
                    xn_tile(b, ci, j),
                    i128,
                    start=True,
                    stop=True,
                )
            dst = xts[:, D * j : D * (j + 1)]
            if j % 2 == 0:
                nc.scalar.copy(dst, xt[:])
            else:
                nc.vector.tensor_copy(dst, xt[:])

    def stage_exp(b):
        """exp(dotsT) -> attnT directly (shift folded into peqT)."""
        attnT = attn_pool.tile([128, 8 * NJ], BF16, name=f"attnT{b}", tag="attnT")
        # exp(dots - 8): 8 is a safe upper bound on the logits (observed max
        # ~4.3), so no max-reduce is needed; the shift cancels in
        # normalization.  Applied via fp32 bias (folding it into bf16 peqT
        # costs ~0.016 absolute per logit).
        nc.scalar.activation(
            attnT[:], state[b]["dt"][:, 0 : 8 * NJ],
            mybir.ActivationFunctionType.Exp,
            bias=nbias[:],
        )
        state[b]["attnT"] = attnT

    def stage_ssum(b):
        """softmax denominators via 1-col matmuls against ones."""
        dt, attnT = state[b]["dt"], state[b]["attnT"]
        for j in range(NJ):
            nc.tensor.matmul(
                dt[0:8, 64:65],
                attnT[:, 8 * j : 8 * j + 8],
                ones[:],
                start=(j == 0),
                stop=(j == NJ - 1),
            )

    def stage_rsum(b):
        # recips collect as columns of rs8 (engines can't write at a
        # partition offset); a tiny SBUF->SBUF DMA on the otherwise-idle
        # gpsimd ring scatters each into rsum_hb's strided rows (parking
        # there is free).  The last batch goes via SP in the tail.
        nc.vector.reciprocal(rs8[:, b : b + 1], state[b]["dt"][0:8, 64:65])
        if b < BPC - 1:
            nc.gpsimd.dma_start(rsv[:, b : b + 1, :], rs8[:, b : b + 1])

    def stage_pool(b):
        """sT[c, (ci,b,h)] += xT_tile^T @ attnT — 8-col matmuls, x stationary."""
        xts, attnT = state[b]["xts"], state[b]["attnT"]
        for ci in range(NCI):
            o = st_acc[:, 64 * ci + 8 * b : 64 * ci + 8 * b + 8]
            for j in range(NJ):
                nc.tensor.matmul(
                    o,
                    xts[:, D * j + 128 * ci : D * j + 128 * ci + 128],
                    attnT[:, 8 * j : 8 * j + 8],
                    start=(j == 0),
                    stop=(j == NJ - 1),
                )
        del state[b]

    st3 = st_acc[:].rearrange("p (ci q) -> p ci q", q=64)
    sb3 = stsb[:].rearrange("p (ci q) -> p ci q", q=64)

    def stage_tail():
        nc.sync.dma_start(rsv[:, BPC - 1 : BPC, :], rs8[:, BPC - 1 : BPC])
        nc.scalar.copy(stsb[:], st_acc[:])
        # Only the per-head diagonal block of s @ Wv is needed, and each
        # head's rows share the same Wv columns — so compute it TRANSPOSED
        # (stationary Wv tile, moving strided s columns): 256 moving columns
        # instead of 2048, output a tiny [64, 64].
        otp = dt_ps.tile([64, 8 * NH], F32, name="otp", tag="dt")
        sb4 = stsb[:].rearrange("p (ci b h) -> p ci b h", b=BPC, h=8)
        for h in range(NH):
            o = otp[:, 8 * h : 8 * h + 8]
            for ci in range(NCI):
                nc.tensor.matmul(
                    o,
                    wv[:, D * ci + 64 * h : D * ci + 64 * h + 64],
                    sb4[:, ci, :, h : h + 1],  # b = 0..7 of head h, stride 8
                    start=(ci == 0),
                    stop=(ci == NCI - 1),
                )
        nc.vector.tensor_copy(otsb[:], otp[:])
        # transpose [64 d, (h,b)] -> [(h,b), 64 d] in fp32 (exact)
        opsT = dt_ps.tile([64, 64], F32, name="opsT", tag="dt")
        nc.tensor.matmul(opsT[:], otsb[:], i64f[:], start=True, stop=True)
        # out = opsT * (1/sum) + bv, fused on DVE; rows are (h, b)
        nc.vector.scalar_tensor_tensor(
            osb[:], opsT[:], rsum_hb[:], bvh[:],
            mybir.AluOpType.mult, mybir.AluOpType.add,
        )
        nc.sync.dma_start(t["out"], osb[:])

    # software pipeline; stage k of batch b emitted in iteration b + OFF[k]
    for i in range(BPC + 2):
        if i < BPC:
            stage_load(i)
        if i == BPC - 1:
            # queued right behind the last x load, so the transfers slot in
            # as soon as the x stream drains
            nc.sync.dma_start(wv[:], t["wv"])
            nc.sync.dma_start(i64f[:], t["i64f"])
            nc.sync.dma_start(bvh[:], t["bvh"])
        if 1 <= i <= BPC:
            b = i - 1
            # half 0 compute first so Act/DVE copies start while half 1 of
            # the batch is still in flight on the DMA ring
            stage_dots_half(b, 0)
            stage_trans_half(b, 0)
        if 2 <= i <= BPC + 1:
            # placed mid-queue: its copies finished last iteration, and it
            # no longer blocks the next batch's half-0 PE work
            stage_pool(i - 2)
        if 1 <= i <= BPC:
            stage_dots_half(b, 1)
            if b == BPC - 1:
                # last batch: exp/recip slot between the copies so neither
                # the attnT nor the rsum scatter gates the tail
                stage_trans_half(b, 1, upto=1)
                stage_exp(b)
                stage_ssum(b)
                stage_rsum(b)
                stage_trans_half(b, 1, skip=1)
            else:
                stage_exp(b)
                stage_trans_half(b, 1)
                stage_ssum(b)
                stage_rsum(b)
        if i == BPC + 1:
            stage_tail()


_BUILT = None


def _build():
    global _BUILT
    if _BUILT is not None:
        return _BUILT
    nc = bacc.Bacc("TRN2", target_bir_lowering=False, debug=False)
    t = {
        "xb": nc.dram_tensor("xb", (BPC * 4 * D, N // 4), BF16, kind="ExternalInput").ap(),
        "cpack": nc.dram_tensor("cpack", (128, 8 * NCI + 8 * NJ + 128), BF16, kind="ExternalInput").ap(),
        "wv": nc.dram_tensor("wv", (128, NCI * D), BF16, kind="ExternalInput").ap(),
        "i64f": nc.dram_tensor("i64f", (64, 64), F32, kind="ExternalInput").ap(),
        "bvh": nc.dram_tensor("bvh", (64, 64), F32, kind="ExternalInput").ap(),
        "out": nc.dram_tensor("out", (64, 64), F32, kind="ExternalOutput").ap(),
    }
    with tile.TileContext(nc) as tc:
        with ExitStack() as ctx:
            _emit(ctx, tc, t)
    nc.compile()
    _BUILT = (nc, t)
    return _BUILT


def _host_consts(q, Wkv, bkv):
    qh = np.asarray(q, np.float32)[0, :, 0, :]                      # (8, 64)
    Wk = np.asarray(Wkv, np.float32)[:, :D]
    Wv = np.asarray(Wkv, np.float32)[:, D:]
    bv = np.asarray(bkv, np.float32)[D:]

    position = np.arange(N, dtype=np.float32)[:, None]
    div_term = np.exp(
        np.arange(0, DH, 2, dtype=np.float32) * (-(math.log(10000.0) / DH))
    )
    pe = np.zeros((N, DH), np.float32)
    pe[:, 0::2] = np.sin(position * div_term)
    pe[:, 1::2] = np.cos(position * div_term)

    wq = np.einsum("chd,hd->ch", Wk.reshape(D, NH, DH), qh) * SCALE  # (512, 8)
    peq = pe @ (qh * SCALE).T                                        # (1024, 8)

    wqpe = np.zeros((128, 8 * NCI), np.float32)
    for ci in range(NCI):
        wqpe[:, 8 * ci : 8 * ci + 8] = wq[128 * ci : 128 * (ci + 1), :]
    peqT = np.zeros((128, 8 * NJ), np.float32)
    for j in range(NJ):
        peqT[:, 8 * j : 8 * j + 8] = peq[128 * j : 128 * (j + 1), :]

    wv_packed = np.zeros((128, NCI * D), np.float32)
    for ci in range(NCI):
        wv_packed[:, D * ci : D * (ci + 1)] = Wv[128 * ci : 128 * (ci + 1), :]

    cpack = np.concatenate(
        [wqpe, peqT, np.eye(128, dtype=np.float32)], axis=1
    )
    # bias rows in (h, b) order: row 8h+b gets bv[64h : 64h+64]
    bvh = np.repeat(bv.reshape(NH, DH), BPC, axis=0).astype(np.float32)
    return {
        "cpack": cpack.astype(ml_dtypes.bfloat16),
        "wv": wv_packed.astype(ml_dtypes.bfloat16),
        "i64f": np.eye(64, dtype=np.float32),
        "bvh": bvh,
    }


def kernel(x, q, Wkv, bkv, num_heads, **kw):
    assert int(num_heads) == NH
    nc, _ = _build()
    consts = _host_consts(q, Wkv, bkv)

    # device layout (b, q, ci, p, nn): a half-batch load is one contiguous
    # SBUF column range with a 3-dim src access pattern
    xb = np.asarray(x, np.float32).reshape(B, NCI, 128, 4, N // 4)
    xb = xb.transpose(0, 3, 1, 2, 4).astype(ml_dtypes.bfloat16)

    in_maps = []
    for i in range(NCORES):
        m = dict(consts)
        m["xb"] = np.ascontiguousarray(xb[i * BPC : (i + 1) * BPC]).reshape(
            BPC * 4 * D, N // 4
        )
        in_maps.append(m)

    res = run_bass_kernel_spmd(nc, in_maps, core_ids=list(range(NCORES)))

    out = np.zeros((B, NH * DH), np.float32)
    for i in range(NCORES):
        # device rows are (h, b); col d is head-h's 64-wide output slice
        shard = res.results[i]["out"].reshape(NH, BPC, DH).transpose(1, 0, 2)
        out[i * BPC : (i + 1) * BPC] = shard.reshape(BPC, NH * DH)
    return out


if __name__ == "__main__":
    _build()
    print("build ok")

